# revision 1
# baseline (speedup 1.0000x reference)
"""EpisodicMemory kernel for Trainium2, data-parallel over batch on 8 NeuronCores.

Per-core computation (one batch element b, S=4096, D=1024, M=64, H=4, DH=256):

Host-side algebraic fusion (exact linear algebra, done in fp32/fp64 numpy):
  k        = mk @ wk.T + bk                              (M, D)
  FUSED_K  = stack_h[(k_h @ wq_h) / sqrt(DH)]            (H*M, D)
  scores   = x @ FUSED_K.T + sbias        (replaces q-proj + qk matmul)
  BIG_W    = [mk | wg | FUSED_K]                         (M+1+H*M, D)
  fused2   = comb_w[:, D:] @ out_w                       (D, D)
  FINAL_W  = [comb_w[:, :D] | fused2]                    (D, 2D)
  combb    = comb_b + comb_w[:, D:] @ out_b              (D,)

Device phases (per core):
  1. per s-chunk (128 rows): pbig = x_chunk @ BIG_W.T -> [sim | gate | scores]
     softmax(sim)*sigmoid(gate) -> gated -> accumulate W = gated.T @ [x | 1]
     softmax(scores) per head -> p -> PE-transpose -> pT_all (resident)
  2. slot_gate = min(colsum, 1), memory_values = slot_gate * W
     v = mv @ wv.T + bv  (via PE transpose of mv)
  3. per s-tile (512): mem_out^T = v.T-slices @ pT_all, then
     final = [x | mem_out] @ FINAL_W.T + combb -> DMA out
All matmuls run in bf16 inputs with fp32 PSUM accumulation.
"""

import numpy as np
import ml_dtypes

import concourse.bass as bass
import concourse.mybir as mybir
import concourse.tile as tile
from concourse import bacc
from concourse.bass_utils import run_bass_kernel_spmd
from concourse.masks import make_identity

F32 = mybir.dt.float32
BF16 = mybir.dt.bfloat16
AX = mybir.AxisListType.X
AF = mybir.ActivationFunctionType

B, D, M, H = 8, 1024, 64, 4
DH = D // H
GW = M + 1 + H * M  # 321 columns of BIG_W output
N_CORES = 8


def build_program(S=4096, add_sbias=False):
    NCH = S // 128   # s-chunks
    NT = S // 512    # s-tiles
    DC = D // 128    # d-chunks

    nc = bacc.Bacc(None, target_bir_lowering=False, debug=False)

    x_d = nc.dram_tensor("x", [S, D], BF16, kind="ExternalInput")
    xT_d = nc.dram_tensor("xT", [D, S], BF16, kind="ExternalInput")
    bigwT_d = nc.dram_tensor("bigwT", [D, GW], BF16, kind="ExternalInput")
    wvT_d = nc.dram_tensor("wvT", [D, D], BF16, kind="ExternalInput")
    fwT_d = nc.dram_tensor("fwT", [2 * D, D], BF16, kind="ExternalInput")
    bv_d = nc.dram_tensor("bv", [D], F32, kind="ExternalInput")
    combb_d = nc.dram_tensor("combb", [D], F32, kind="ExternalInput")
    wgb_d = nc.dram_tensor("wgb", [1], F32, kind="ExternalInput")
    sbias_d = nc.dram_tensor("sbias", [H * M], F32, kind="ExternalInput")
    y_d = nc.dram_tensor("y", [S, D], F32, kind="ExternalOutput")

    x_ap = x_d.ap()
    y_ap = y_d.ap()
    xT_r = xT_d.ap().rearrange("(dc p) s -> p dc s", p=128)
    bigwT_r = bigwT_d.ap().rearrange("(dc p) g -> p dc g", p=128)
    wvT_r = wvT_d.ap().rearrange("(dc p) g -> p dc g", p=128)
    fwT_r = fwT_d.ap().rearrange("(cc p) g -> p cc g", p=128)

    def bcast(ap, n):
        return bass.AP(tensor=ap.tensor, offset=ap.offset, ap=[[0, n]] + list(ap.ap))

    with tile.TileContext(nc) as tc:
        with tc.tile_pool(name="singles", bufs=1) as singles:
            bigwT_sb = singles.tile([128, DC, GW], BF16)
            nc.sync.dma_start(bigwT_sb, bigwT_r)
            wvT_sb = singles.tile([128, DC, D], BF16)
            nc.sync.dma_start(wvT_sb, wvT_r)
            fwT_sb = singles.tile([128, 2 * DC, D], BF16)
            nc.sync.dma_start(fwT_sb, fwT_r)
            combb_sb = singles.tile([128, D], F32)
            nc.gpsimd.dma_start(combb_sb, bcast(combb_d.ap(), 128))
            bvb_sb = singles.tile([64, D], F32)
            nc.gpsimd.dma_start(bvb_sb, bcast(bv_d.ap(), 64))
            wgb_sb = singles.tile([128, 1], F32)
            nc.gpsimd.dma_start(wgb_sb, bcast(wgb_d.ap(), 128))
            sbias_sb = singles.tile([128, H * M], F32)
            nc.gpsimd.dma_start(sbias_sb, bcast(sbias_d.ap(), 128))
            ident = singles.tile([128, 128], BF16)
            make_identity(nc, ident)
            ones_sb = singles.tile([128, 1], BF16)
            nc.vector.memset(ones_sb, 1.0)
            pT_all = singles.tile([128, 2, S], BF16)

            # ---------------- phase 1: write-attention ----------------
            with (
                tc.tile_pool(name="ps1", bufs=1, space="PSUM") as ps1,
                tc.tile_pool(name="xin", bufs=2) as xin,
                tc.tile_pool(name="wk1", bufs=2) as wk1,
            ):
                ps_w = ps1.tile([64, 1536], F32, tag="w")
                for t in range(NT):
                    xtile = xin.tile([128, DC, 512], BF16, tag="xt")
                    nc.sync.dma_start(xtile, xT_r[:, :, t * 512:(t + 1) * 512])
                    for c4 in range(4):
                        c = t * 4 + c4
                        xc = xin.tile([128, D], BF16, tag="xc", bufs=3)
                        nc.sync.dma_start(xc, x_ap[c * 128:(c + 1) * 128, :])

                        pbig = ps1.tile([128, GW], F32, tag="big", bufs=2)
                        for dc in range(DC):
                            nc.tensor.matmul(
                                pbig,
                                lhsT=xtile[:, dc, c4 * 128:(c4 + 1) * 128],
                                rhs=bigwT_sb[:, dc, :],
                                start=(dc == 0), stop=(dc == DC - 1),
                            )
                        if add_sbias:
                            nc.vector.tensor_add(
                                pbig[:, M + 1:GW], pbig[:, M + 1:GW], sbias_sb
                            )

                        # --- write gate: softmax(sim) * sigmoid(gate) ---
                        nmx = wk1.tile([128, 1], F32, tag="nmx")
                        nc.vector.reduce_max(nmx, pbig[:, 0:M], axis=AX, negate=True)
                        esum = wk1.tile([128, 1], F32, tag="esum")
                        e_sb = wk1.tile([128, M], F32, tag="esb")
                        nc.scalar.activation(e_sb, pbig[:, 0:M], AF.Exp,
                                             bias=nmx, accum_out=esum)
                        gate = wk1.tile([128, 1], F32, tag="gate")
                        nc.scalar.activation(gate, pbig[:, M:M + 1], AF.Sigmoid,
                                             bias=wgb_sb)
                        rsum = wk1.tile([128, 1], F32, tag="rsum")
                        nc.vector.reciprocal(rsum, esum)
                        scale = wk1.tile([128, 1], F32, tag="scale")
                        nc.vector.tensor_mul(scale, gate, rsum)
                        gc = wk1.tile([128, M], BF16, tag="gc")
                        nc.vector.tensor_scalar_mul(gc, e_sb, scale)

                        # --- read attention probs, per head ---
                        pn = wk1.tile([128, H * M], BF16, tag="pn")
                        for h in range(H):
                            c0 = M + 1 + h * M
                            nmxh = wk1.tile([128, 1], F32, tag="nmxh")
                            nc.vector.reduce_max(nmxh, pbig[:, c0:c0 + M],
                                                 axis=AX, negate=True)
                            esh = wk1.tile([128, 1], F32, tag="esh")
                            eh = wk1.tile([128, M], F32, tag="eh")
                            nc.scalar.activation(eh, pbig[:, c0:c0 + M], AF.Exp,
                                                 bias=nmxh, accum_out=esh)
                            rsh = wk1.tile([128, 1], F32, tag="rsh")
                            nc.vector.reciprocal(rsh, esh)
                            nc.vector.tensor_scalar_mul(
                                pn[:, h * M:(h + 1) * M], eh, rsh)

                        for j2 in range(2):
                            ptr = ps1.tile([128, 128], BF16, tag="tr", bufs=2)
                            nc.tensor.transpose(
                                ptr, pn[:, j2 * 128:(j2 + 1) * 128], ident)
                            nc.vector.tensor_copy(
                                pT_all[:, j2, c * 128:(c + 1) * 128], ptr)

                        # --- accumulate W = gated.T @ [x | 1] ---
                        st, sp = (c == 0), (c == NCH - 1)
                        nc.tensor.matmul(ps_w[:, 0:512], lhsT=gc,
                                         rhs=xc[:, 0:512], start=st, stop=sp)
                        nc.tensor.matmul(ps_w[:, 512:1024], lhsT=gc,
                                         rhs=xc[:, 512:1024], start=st, stop=sp)
                        nc.tensor.matmul(ps_w[:, 1024:1025], lhsT=gc,
                                         rhs=ones_sb, start=st, stop=sp)

                # --- slot gate ---
                ssum = singles.tile([64, 1], F32)
                nc.vector.tensor_copy(ssum, ps_w[:, 1024:1025])
                sg = singles.tile([64, 1], F32)
                nc.vector.tensor_scalar_min(sg, ssum, 1.0)
                mv_bf = singles.tile([64, D], BF16)
                nc.vector.tensor_scalar_mul(mv_bf, ps_w[:, 0:D], sg)

            # ---------------- phase boundary: v projection ----------------
            mvT_sb = singles.tile([128, DC, 64], BF16)
            v_sb = singles.tile([64, D], BF16)
            v2_sb = singles.tile([128, D], BF16)
            with tc.tile_pool(name="psB", bufs=1, space="PSUM") as psB:
                for dc in range(DC):
                    ptr2 = psB.tile([128, 128], BF16, tag="tr2", bufs=2)
                    nc.tensor.transpose(
                        ptr2[:, 0:64],
                        mv_bf[:, dc * 128:(dc + 1) * 128],
                        ident[0:64, 0:64],
                    )
                    nc.vector.tensor_copy(mvT_sb[:, dc, :], ptr2[:, 0:64])
                pv = psB.tile([64, D], F32, tag="v")
                for g2 in range(2):
                    for dc in range(DC):
                        nc.tensor.matmul(
                            pv[:, g2 * 512:(g2 + 1) * 512],
                            lhsT=mvT_sb[:, dc, :],
                            rhs=wvT_sb[:, dc, g2 * 512:(g2 + 1) * 512],
                            start=(dc == 0), stop=(dc == DC - 1),
                        )
                nc.vector.tensor_add(v_sb, pv, bvb_sb)
            nc.sync.dma_start(v2_sb[0:64, :], v_sb)
            nc.sync.dma_start(v2_sb[64:128, :], v_sb)

            # ---------------- phase 2: read attention + output ----------------
            with (
                tc.tile_pool(name="ps2", bufs=1, space="PSUM") as ps2,
                tc.tile_pool(name="xin2", bufs=2) as xin2,
                tc.tile_pool(name="wk2", bufs=2) as wk2,
            ):
                for t in range(NT):
                    xtile = xin2.tile([128, DC, 512], BF16, tag="xt")
                    nc.sync.dma_start(xtile, xT_r[:, :, t * 512:(t + 1) * 512])
                    moT = wk2.tile([128, DC, 512], BF16, tag="mo")
                    for ec in range(DC):
                        h = ec // 2
                        poff = 64 * (h % 2)
                        j2 = h // 2
                        pmo = ps2.tile([128, 512], F32, tag="mo", bufs=2)
                        nc.tensor.matmul(
                            pmo,
                            lhsT=v2_sb[poff:poff + 64, ec * 128:(ec + 1) * 128],
                            rhs=pT_all[poff:poff + 64, j2, t * 512:(t + 1) * 512],
                            start=True, stop=True,
                        )
                        nc.scalar.copy(moT[:, ec, :], pmo)
                    for c4 in range(4):
                        for gh in range(2):
                            pf = ps2.tile([128, 512], F32, tag="f", bufs=2)
                            for dc in range(DC):
                                nc.tensor.matmul(
                                    pf,
                                    lhsT=xtile[:, dc, c4 * 128:(c4 + 1) * 128],
                                    rhs=fwT_sb[:, dc, gh * 512:(gh + 1) * 512],
                                    start=(dc == 0), stop=False,
                                )
                            for fc in range(DC):
                                nc.tensor.matmul(
                                    pf,
                                    lhsT=moT[:, fc, c4 * 128:(c4 + 1) * 128],
                                    rhs=fwT_sb[:, DC + fc, gh * 512:(gh + 1) * 512],
                                    start=False, stop=(fc == DC - 1),
                                )
                            osb = wk2.tile([128, 512], F32, tag="osb", bufs=3)
                            nc.vector.tensor_add(
                                osb, pf, combb_sb[:, gh * 512:(gh + 1) * 512])
                            nc.sync.dma_start(
                                y_ap[(t * 4 + c4) * 128:(t * 4 + c4 + 1) * 128,
                                     gh * 512:(gh + 1) * 512],
                                osb,
                            )

    nc.compile()
    return nc


def prep_inputs(inputs, S=4096):
    """Host-side fusion + per-core shard maps."""
    f64 = np.float64
    bf = ml_dtypes.bfloat16
    x = np.asarray(inputs["x"], np.float32)
    mk = np.asarray(inputs["memory_keys"], np.float32)
    wg_w = np.asarray(inputs["wg_w"], np.float32)
    wg_b = np.asarray(inputs["wg_b"], np.float32)
    ipw = np.asarray(inputs["in_proj_w"], np.float32)
    ipb = np.asarray(inputs["in_proj_b"], np.float32)
    out_w = np.asarray(inputs["out_w"], np.float32)
    out_b = np.asarray(inputs["out_b"], np.float32)
    comb_w = np.asarray(inputs["comb_w"], np.float32)
    comb_b = np.asarray(inputs["comb_b"], np.float32)

    wq, wk, wv = ipw[:D], ipw[D:2 * D], ipw[2 * D:]
    bq, bk, bv = ipb[:D], ipb[D:2 * D], ipb[2 * D:]

    k_full = mk.astype(f64) @ wk.astype(f64).T + bk.astype(f64)      # (M, D)
    kh = k_full.reshape(M, H, DH)
    wqh = wq.astype(f64).reshape(H, DH, D)
    scl = 1.0 / np.sqrt(DH)
    FK = (np.einsum("mhd,hde->hme", kh, wqh) * scl).reshape(H * M, D)
    sbias = (np.einsum("hd,mhd->hm", bq.astype(f64).reshape(H, DH), kh)
             * scl).reshape(H * M)
    BIG_W = np.concatenate([mk.astype(f64), wg_w.astype(f64), FK], axis=0)

    fused2 = comb_w[:, D:].astype(f64) @ out_w.astype(f64)           # (D, D)
    FINAL_W = np.concatenate([comb_w[:, :D].astype(f64), fused2], axis=1)
    combb = comb_b.astype(f64) + comb_w[:, D:].astype(f64) @ out_b.astype(f64)

    shared = {
        "bigwT": np.ascontiguousarray(BIG_W.T).astype(bf),
        "wvT": np.ascontiguousarray(wv.T).astype(bf),
        "fwT": np.ascontiguousarray(FINAL_W.T).astype(bf),
        "bv": bv.astype(np.float32),
        "combb": combb.astype(np.float32),
        "wgb": wg_b.astype(np.float32),
        "sbias": sbias.astype(np.float32),
    }
    add_sbias = bool(np.any(shared["sbias"] != 0))

    in_maps = []
    for b in range(B):
        xb = x[b, :S]
        m = dict(shared)
        m["x"] = xb.astype(bf)
        m["xT"] = np.ascontiguousarray(xb.T).astype(bf)
        in_maps.append(m)
    return in_maps, add_sbias


def kernel(_trace=False, _S=4096, **inputs):
    in_maps, add_sbias = prep_inputs(inputs, S=_S)
    nc = build_program(S=_S, add_sbias=add_sbias)
    kw = {}
    if _trace:
        kw = dict(trace=True, trace_cores=list(range(N_CORES)))
    res = run_bass_kernel_spmd(nc, in_maps, list(range(N_CORES)), **kw)
    y = np.stack([res.results[i]["y"] for i in range(N_CORES)], axis=0)
    y = y.astype(np.float32)
    if _trace:
        return y, res
    return y



# revision 5
# speedup vs baseline: 1.4570x; 1.4570x over previous
"""EpisodicMemory kernel for Trainium2, data-parallel over batch on 8 NeuronCores.

Per-core computation (one batch element b, S=4096, D=1024, M=64, H=4, DH=256):

Host-side algebraic fusion (exact linear algebra, fp64 numpy):
  k        = mk @ wk.T + bk                              (M, D)
  FUSED_K  = stack_h[(k_h @ wq_h) / sqrt(DH)]            (H*M, D)
  scores   = x @ FUSED_K.T + sbias        (replaces q-proj + qk matmul)
  BIG_W    = [mk | wg | FUSED_K]                         (M+1+H*M, D)
  fused2   = comb_w[:, D:] @ out_w                       (D, D)
  cw1      = comb_w[:, :D]                               (D, D)
  combb    = comb_b + comb_w[:, D:] @ out_b              (D,)

Device algebra: instead of materializing mem_out (S, D) and contracting
over 2D, fold fused2 into the value path per head:
  VF[(h,m), :] = v[m, hDH:(h+1)DH] @ fused2[:, hDH:(h+1)DH].T   (H*M, D)
  y = x @ cw1.T + P @ VF + combb     where P = concat_h softmax_h(scores)

Device phases (per core):
  1. per s-chunk (128 rows): pbig = x_chunk @ BIG_W.T -> [sim | gate | scores]
     exp / sigmoid (logits are tiny -> no max subtraction), gated write probs,
     accumulate W = gated.T @ [x | 1], read-probs P -> PE-transpose -> pT_all.
  2. slot_gate = min(colsum, 1); mv = slot_gate * W; v = mv @ wv.T + bv;
     VF per head via PE transposes + small matmuls.
  3. transposed output: for each d-chunk, yT[d, :] accumulates
     cw1T-chunks.T @ xT-stream + VF-chunks.T @ pT-stream in PSUM,
     + combb, written bf16 (host transposes back).
All matmuls bf16 inputs with fp32 PSUM accumulation.
"""

import numpy as np
import ml_dtypes

import concourse.bass as bass
import concourse.mybir as mybir
import concourse.tile as tile
from concourse import bacc
from concourse.bass_utils import run_bass_kernel_spmd
from concourse.masks import make_identity

F32 = mybir.dt.float32
BF16 = mybir.dt.bfloat16
AX = mybir.AxisListType.X
AF = mybir.ActivationFunctionType

B, D, M, H = 8, 1024, 64, 4
DH = D // H
GW = M + 1 + H * M  # 321 columns of BIG_W output
N_CORES = 8


def build_program(S=4096, add_sbias=False):
    NCH = S // 128   # s-chunks
    DC = D // 128    # d-chunks

    nc = bacc.Bacc(None, target_bir_lowering=False, debug=False)

    x_d = nc.dram_tensor("x", [S, D], BF16, kind="ExternalInput")
    xT_d = nc.dram_tensor("xT", [D, S], BF16, kind="ExternalInput")
    bigwT_d = nc.dram_tensor("bigwT", [D, GW], BF16, kind="ExternalInput")
    wvT_d = nc.dram_tensor("wvT", [D, D], BF16, kind="ExternalInput")
    f2T_d = nc.dram_tensor("f2T", [D, D], BF16, kind="ExternalInput")
    cw1T_d = nc.dram_tensor("cw1T", [D, D], BF16, kind="ExternalInput")
    bv_d = nc.dram_tensor("bv", [D], F32, kind="ExternalInput")
    combb_d = nc.dram_tensor("combb", [D], F32, kind="ExternalInput")
    wgb_d = nc.dram_tensor("wgb", [1], F32, kind="ExternalInput")
    sbias_d = nc.dram_tensor("sbias", [H * M], F32, kind="ExternalInput")
    yT_d = nc.dram_tensor("yT", [D, S], BF16, kind="ExternalOutput")

    x_ap = x_d.ap()
    yT_ap = yT_d.ap()
    xT_r = xT_d.ap().rearrange("(dc p) s -> p dc s", p=128)
    bigwT_r = bigwT_d.ap().rearrange("(dc p) g -> p dc g", p=128)
    wvT_r = wvT_d.ap().rearrange("(dc p) g -> p dc g", p=128)
    f2T_r = f2T_d.ap().rearrange("(dc p) g -> p dc g", p=128)
    cw1T_r = cw1T_d.ap().rearrange("(dc p) g -> p dc g", p=128)
    combb_r = combb_d.ap().rearrange("(dc p) -> p dc", p=128)

    def bcast(ap, n):
        return bass.AP(tensor=ap.tensor, offset=ap.offset, ap=[[0, n]] + list(ap.ap))

    with tile.TileContext(nc) as tc:
        with tc.tile_pool(name="singles", bufs=1) as singles:
            xT_sb = singles.tile([128, DC, S], BF16)
            nc.sync.dma_start(xT_sb, xT_r)
            bigwT_sb = singles.tile([128, DC, GW], BF16)
            nc.sync.dma_start(bigwT_sb, bigwT_r)
            cw1T_sb = singles.tile([128, DC, D], BF16)
            nc.sync.dma_start(cw1T_sb, cw1T_r)
            wvT_sb = singles.tile([128, DC, D], BF16)
            nc.sync.dma_start(wvT_sb, wvT_r)
            f2T_sb = singles.tile([128, DC, D], BF16)
            nc.sync.dma_start(f2T_sb, f2T_r)
            combb_sb = singles.tile([128, DC], F32)
            nc.sync.dma_start(combb_sb, combb_r)
            bvb_sb = singles.tile([64, D], F32)
            nc.gpsimd.dma_start(bvb_sb, bcast(bv_d.ap(), 64))
            wgb_sb = singles.tile([128, 1], F32)
            nc.gpsimd.dma_start(wgb_sb, bcast(wgb_d.ap(), 128))
            sbias_sb = singles.tile([128, H * M], F32)
            nc.gpsimd.dma_start(sbias_sb, bcast(sbias_d.ap(), 128))
            ident = singles.tile([128, 128], BF16)
            make_identity(nc, ident)
            ones_sb = singles.tile([128, 1], BF16)
            nc.vector.memset(ones_sb, 1.0)
            pT_all = singles.tile([128, 2, S], BF16)

            # ---------------- phase 1: write-attention ----------------
            with (
                tc.tile_pool(name="ps1", bufs=1, space="PSUM") as ps1,
                tc.tile_pool(name="xin", bufs=3) as xin,
                tc.tile_pool(name="wk1", bufs=2) as wk1,
            ):
                ps_w = ps1.tile([64, 1536], F32, tag="w")

                def issue_pbig(c):
                    pbig = ps1.tile([128, GW], F32, tag="big", bufs=2)
                    for dc in range(DC):
                        nc.tensor.matmul(
                            pbig,
                            lhsT=xT_sb[:, dc, c * 128:(c + 1) * 128],
                            rhs=bigwT_sb[:, dc, :],
                            start=(dc == 0), stop=(dc == DC - 1),
                        )
                    return pbig

                def process(c, pbig, xc):
                    if add_sbias:
                        nc.vector.tensor_add(
                            pbig[:, M + 1:GW], pbig[:, M + 1:GW], sbias_sb
                        )
                    # --- write gate: softmax(sim) * sigmoid(gate) ---
                    esum = wk1.tile([128, 1], F32, tag="esum")
                    e_sb = wk1.tile([128, M], F32, tag="esb")
                    nc.scalar.activation(e_sb, pbig[:, 0:M], AF.Exp,
                                         accum_out=esum)
                    gate = wk1.tile([128, 1], F32, tag="gate")
                    nc.scalar.activation(gate, pbig[:, M:M + 1], AF.Sigmoid,
                                         bias=wgb_sb)
                    rsum = wk1.tile([128, 1], F32, tag="rsum")
                    nc.vector.reciprocal(rsum, esum)
                    scale = wk1.tile([128, 1], F32, tag="scale")
                    nc.vector.tensor_mul(scale, gate, rsum)
                    gc = wk1.tile([128, M], BF16, tag="gc")
                    nc.vector.tensor_scalar_mul(gc, e_sb, scale)

                    # --- read attention probs, all 4 heads at once ---
                    eh = wk1.tile([128, H, M], F32, tag="eh")
                    nc.scalar.activation(eh, pbig[:, M + 1:GW], AF.Exp)
                    hs = wk1.tile([128, H], F32, tag="hs")
                    nc.vector.reduce_sum(hs, eh, axis=AX)
                    rh = wk1.tile([128, H], F32, tag="rh")
                    nc.vector.reciprocal(rh, hs)
                    pn = wk1.tile([128, H, M], BF16, tag="pn")
                    nc.vector.tensor_mul(pn, eh, rh.broadcast_to((128, H, M)))

                    pn2 = pn.rearrange("p h m -> p (h m)")
                    for j2 in range(2):
                        ptr = ps1.tile([128, 128], BF16, tag="tr", bufs=2)
                        nc.tensor.transpose(
                            ptr, pn2[:, j2 * 128:(j2 + 1) * 128], ident)
                        nc.scalar.copy(
                            pT_all[:, j2, c * 128:(c + 1) * 128], ptr)

                    # --- accumulate W = gated.T @ [x | 1] ---
                    st, sp = (c == 0), (c == NCH - 1)
                    nc.tensor.matmul(ps_w[:, 0:512], lhsT=gc,
                                     rhs=xc[:, 0:512], start=st, stop=sp)
                    nc.tensor.matmul(ps_w[:, 512:1024], lhsT=gc,
                                     rhs=xc[:, 512:1024], start=st, stop=sp)
                    nc.tensor.matmul(ps_w[:, 1024:1025], lhsT=gc,
                                     rhs=ones_sb, start=st, stop=sp)

                prev = None
                for c in range(NCH):
                    xc = xin.tile([128, D], BF16, tag="xc")
                    nc.sync.dma_start(xc, x_ap[c * 128:(c + 1) * 128, :])
                    pbig = issue_pbig(c)
                    if prev is not None:
                        process(*prev)
                    prev = (c, pbig, xc)
                process(*prev)

                # --- slot gate ---
                ssum = singles.tile([64, 1], F32)
                nc.vector.tensor_copy(ssum, ps_w[:, 1024:1025])
                sg = singles.tile([64, 1], F32)
                nc.vector.tensor_scalar_min(sg, ssum, 1.0)
                mv_bf = singles.tile([64, D], BF16)
                nc.vector.tensor_scalar_mul(mv_bf, ps_w[:, 0:D], sg)

            # ---------------- phase boundary: v and VF ----------------
            mvT_sb = singles.tile([128, DC, 64], BF16)
            vT_sb = singles.tile([128, DC, 64], BF16)
            v_sb = singles.tile([64, D], BF16)
            vf_sb = singles.tile([128, 2, D], BF16)
            with tc.tile_pool(name="psB", bufs=1, space="PSUM") as psB:
                for dc in range(DC):
                    ptr2 = psB.tile([128, 64], BF16, tag="tr2", bufs=2)
                    nc.tensor.transpose(
                        ptr2, mv_bf[:, dc * 128:(dc + 1) * 128],
                        ident[0:64, 0:64])
                    nc.vector.tensor_copy(mvT_sb[:, dc, :], ptr2)
                pv = psB.tile([64, D], F32, tag="v")
                for g2 in range(2):
                    for dc in range(DC):
                        nc.tensor.matmul(
                            pv[:, g2 * 512:(g2 + 1) * 512],
                            lhsT=mvT_sb[:, dc, :],
                            rhs=wvT_sb[:, dc, g2 * 512:(g2 + 1) * 512],
                            start=(dc == 0), stop=(dc == DC - 1),
                        )
                nc.vector.tensor_add(v_sb, pv, bvb_sb)
                for dc in range(DC):
                    ptr3 = psB.tile([128, 64], BF16, tag="tr2", bufs=2)
                    nc.tensor.transpose(
                        ptr3, v_sb[:, dc * 128:(dc + 1) * 128],
                        ident[0:64, 0:64])
                    nc.vector.tensor_copy(vT_sb[:, dc, :], ptr3)
                for q in range(2):
                    pvf = psB.tile([128, D], F32, tag="vf", bufs=2)
                    for hh in range(2):
                        h = 2 * q + hh
                        for cc in range(2):
                            for g2 in range(2):
                                nc.tensor.matmul(
                                    pvf[hh * 64:(hh + 1) * 64,
                                        g2 * 512:(g2 + 1) * 512],
                                    lhsT=vT_sb[:, h * 2 + cc, :],
                                    rhs=f2T_sb[:, h * 2 + cc,
                                               g2 * 512:(g2 + 1) * 512],
                                    start=(cc == 0), stop=(cc == 1),
                                )
                    nc.vector.tensor_copy(vf_sb[:, q, :], pvf)

            # ---------------- phase 2: transposed output ----------------
            with (
                tc.tile_pool(name="ps2", bufs=2, space="PSUM") as ps2,
                tc.tile_pool(name="wk2", bufs=2) as wk2,
            ):
                for dc in range(DC):
                    for sh in range(2):
                        pts = [ps2.tile([128, 512], F32, tag=f"o{st}",
                                        name=f"pt{st}")
                               for st in range(4)]
                        for ci in range(DC + 2):
                            if ci < DC:
                                lhsT = cw1T_sb[:, ci, dc * 128:(dc + 1) * 128]
                            else:
                                lhsT = vf_sb[:, ci - DC,
                                             dc * 128:(dc + 1) * 128]
                            for st in range(4):
                                s0 = sh * 2048 + st * 512
                                if ci < DC:
                                    rhs = xT_sb[:, ci, s0:s0 + 512]
                                else:
                                    rhs = pT_all[:, ci - DC, s0:s0 + 512]
                                nc.tensor.matmul(
                                    pts[st], lhsT=lhsT, rhs=rhs,
                                    start=(ci == 0), stop=(ci == DC + 1),
                                )
                        ytile = wk2.tile([128, 4, 512], BF16, tag="yt")
                        for st in range(4):
                            nc.vector.tensor_scalar_add(
                                ytile[:, st, :], pts[st],
                                combb_sb[:, dc:dc + 1])
                        nc.sync.dma_start(
                            yT_ap[dc * 128:(dc + 1) * 128,
                                  sh * 2048:(sh + 1) * 2048],
                            ytile,
                        )

    nc.compile()
    return nc


def prep_inputs(inputs, S=4096):
    """Host-side fusion + per-core shard maps."""
    f64 = np.float64
    bf = ml_dtypes.bfloat16
    x = np.asarray(inputs["x"], np.float32)
    mk = np.asarray(inputs["memory_keys"], np.float32)
    wg_w = np.asarray(inputs["wg_w"], np.float32)
    wg_b = np.asarray(inputs["wg_b"], np.float32)
    ipw = np.asarray(inputs["in_proj_w"], np.float32)
    ipb = np.asarray(inputs["in_proj_b"], np.float32)
    out_w = np.asarray(inputs["out_w"], np.float32)
    out_b = np.asarray(inputs["out_b"], np.float32)
    comb_w = np.asarray(inputs["comb_w"], np.float32)
    comb_b = np.asarray(inputs["comb_b"], np.float32)

    wq, wk, wv = ipw[:D], ipw[D:2 * D], ipw[2 * D:]
    bq, bk, bv = ipb[:D], ipb[D:2 * D], ipb[2 * D:]

    k_full = mk.astype(f64) @ wk.astype(f64).T + bk.astype(f64)      # (M, D)
    kh = k_full.reshape(M, H, DH)
    wqh = wq.astype(f64).reshape(H, DH, D)
    scl = 1.0 / np.sqrt(DH)
    FK = (np.einsum("mhd,hde->hme", kh, wqh) * scl).reshape(H * M, D)
    sbias = (np.einsum("hd,mhd->hm", bq.astype(f64).reshape(H, DH), kh)
             * scl).reshape(H * M)
    BIG_W = np.concatenate([mk.astype(f64), wg_w.astype(f64), FK], axis=0)

    fused2 = comb_w[:, D:].astype(f64) @ out_w.astype(f64)           # (D, D)
    combb = comb_b.astype(f64) + comb_w[:, D:].astype(f64) @ out_b.astype(f64)

    shared = {
        "bigwT": np.ascontiguousarray(BIG_W.T).astype(bf),
        "wvT": np.ascontiguousarray(wv.T).astype(bf),
        "f2T": np.ascontiguousarray(fused2.T).astype(bf),
        "cw1T": np.ascontiguousarray(comb_w[:, :D].T).astype(bf),
        "bv": bv.astype(np.float32),
        "combb": combb.astype(np.float32),
        "wgb": wg_b.astype(np.float32),
        "sbias": sbias.astype(np.float32),
    }
    add_sbias = bool(np.any(shared["sbias"] != 0))

    in_maps = []
    for b in range(B):
        xb = x[b, :S]
        m = dict(shared)
        m["x"] = xb.astype(bf)
        m["xT"] = np.ascontiguousarray(xb.T).astype(bf)
        in_maps.append(m)
    return in_maps, add_sbias


def kernel(_trace=False, _S=4096, **inputs):
    in_maps, add_sbias = prep_inputs(inputs, S=_S)
    nc = build_program(S=_S, add_sbias=add_sbias)
    kw = {}
    if _trace:
        kw = dict(trace=True, trace_cores=list(range(N_CORES)))
    res = run_bass_kernel_spmd(nc, in_maps, list(range(N_CORES)), **kw)
    y = np.stack(
        [np.asarray(res.results[i]["yT"]).astype(np.float32).T
         for i in range(N_CORES)],
        axis=0,
    )
    if _trace:
        return y, res
    return y
